# revision 1
# baseline (speedup 1.0000x reference)
"""CapsuleConv2d (k-means routing, 3 iters) Trainium2 Bass kernel.

Problem (hardcoded): x [2,128,32,32] f32, weight [16,16,16,3,3] f32
(w[o,l,m,i,j]), stride 1, pad 1, G=8 groups of M=16 in-channels,
N_in = G*KH*KW = 72 votes, O=16 out-capsules of L=16.
Output [2, 256, 32, 32] f32.

Sharding: data-parallel over (b, oh): 64 rows -> 8 cores x 8 rows.
Each core processes 2 chunks of 128 positions (4 oh-rows x 32 ow).
Host passes each core its own 10-row x-slab (same shapes, different
data -> SPMD program identical across cores).

Per-chunk pipeline (single NeuronCore):
  PE:  priors u[p,(n,l,o)] via 72 fp32r matmuls [K=16(m), M=128(p),
       N=256(l,o)] (lhsT = x-slab window slice, rhs = weight tap) + 72
       accumulating matmuls for v0 = sum_n u (routing init, unnormalized).
  ACT: PSUM->SBUF copies (cast bf16), exp, and rsqrt/sqrt via exp/ln
       (one table set -> no table switches).
  DVE: routing in layout [p partitions, (n, l, o) free]:
       z1 = u*bcast(vn) (bf16 2x) -> reduce_l -> logits (fp32)
       e = exp(logits) -> z2 = u*bcast(e) (bf16 2x) -> reduce_n -> v_u
  final: squash fused with softmax denom: out = v_u*||v_u||/(S^2+||v_u||^2),
       PE-transpose [p,(o,l)] -> [(o,l),p], channel-major output DMA.
"""
from contextlib import ExitStack

import numpy as np

B, CIN, H, W = 2, 128, 32, 32
G, M, O, L = 8, 16, 16, 16
NTAP, NIN = 9, 72
COUT = O * L
NCORES = 8
ROWS_PER_CORE = 8  # (b, oh) rows per core
CHUNK_ROWS = 4
NCHUNK = ROWS_PER_CORE // CHUNK_ROWS
P = 128


def _build_bass():
    import concourse.tile as tile
    from concourse import bacc, hw_specs, masks, mybir

    # The act-table pass greedily picks the first set containing each
    # function, ping-ponging exp_and_others <-> natural_log (2.7us/load).
    # Strip Exp/Ln from every set except the combined one so all our ACT
    # work (Exp, Ln, Copy, Identity) lives in a single table set.
    if not getattr(bacc, "_capsule_act_tables_patched", False):
        _orig_gat = bacc.get_activation_tables

        def _gat(arch):
            t = dict(_orig_gat(arch))
            for name, fns in t.items():
                if name != "natural_log_exp_and_others":
                    t[name] = {f for f in fns if f.name not in ("Exp", "Ln")}
            return t

        bacc.get_activation_tables = _gat
        bacc._capsule_act_tables_patched = True

    fp32 = mybir.dt.float32
    f32r = mybir.dt.float32r
    bf16 = mybir.dt.bfloat16
    AX = mybir.AxisListType
    AF = mybir.ActivationFunctionType

    nc = bacc.Bacc("TRN2", target_bir_lowering=False, debug=False)
    # host-pretransposed, pre-padded slab: xs[m, g, h(10), w(34)]
    xs_d = nc.declare_dram_parameter("xs", [M, G, 10, 34], fp32, isOutput=False)
    # same slab, channel-major: xs2[c=(g,m), h(10), w(34)]
    xs2_d = nc.declare_dram_parameter("xs2", [CIN, 10, 34], fp32,
                                      isOutput=False)
    # host-pretransposed weights: wr[m, (tap, l, o)] = w[o, l, m, i, j]
    w_d = nc.declare_dram_parameter("wgt", [M, NTAP * 256], fp32, isOutput=False)
    out_d = nc.declare_dram_parameter("out", [COUT, ROWS_PER_CORE, W], fp32,
                                      isOutput=True)

    with tile.TileContext(nc) as tc, ExitStack() as ctx:
        const_pool = ctx.enter_context(tc.tile_pool(name="const", bufs=1))
        upool = ctx.enter_context(tc.tile_pool(name="u", bufs=2))
        zpool = ctx.enter_context(tc.tile_pool(name="z", bufs=1))
        small = ctx.enter_context(tc.tile_pool(name="small", bufs=2))
        psum = ctx.enter_context(tc.tile_pool(name="ps", bufs=4, space="PSUM"))
        tpsum = ctx.enter_context(tc.tile_pool(name="tps", bufs=2, space="PSUM"))
        vpsum = ctx.enter_context(tc.tile_pool(name="vps", bufs=2, space="PSUM"))

        # ---- constants (once per core) ----
        # weights replicated over g: wr2[(g,m), (t,l,o)]; wr_r = g=0 slice.
        # wgt DMAs first: the wr2 cast gates the first matmuls.
        wr2_f = const_pool.tile([CIN, NTAP * 256], fp32)
        for g in range(G):
            eng = nc.sync if g % 2 == 0 else nc.scalar
            eng.dma_start(out=wr2_f[g * M:(g + 1) * M, :], in_=w_d[:])
        wr2 = const_pool.tile([CIN, NTAP * 256], f32r)
        nc.vector.tensor_copy(wr2[:], wr2_f[:])
        wr_r = wr2[0:M, :]

        # x slab fp32 [m=16, (g, h=10, w=34)], w-padding included from host
        slab_f = const_pool.tile([M, G * 10 * 34], fp32)
        nc.gpsimd.dma_start(out=slab_f[:],
                            in_=xs_d[:].rearrange("m g h w -> m (g h w)"))
        slab = slab_f[:].rearrange("m (g h w) -> m g h w", g=G, h=10)

        # channel-major slab for the K=128 v0 matmuls
        slab2_f = const_pool.tile([CIN, 10 * 34], fp32)
        nc.gpsimd.dma_start(out=slab2_f[:],
                            in_=xs2_d[:].rearrange("c h w -> c (h w)"))
        slab2 = slab2_f[:].rearrange("c (h w) -> c h w", h=10)

        ident = const_pool.tile([128, 128], fp32)
        masks.make_identity(nc, ident[:])

        # PE warm-up: ~4us of back-to-back dummy matmuls during the initial
        # DMA wait releases the HAM clock throttle before the real matmuls.
        warm = const_pool.tile([128, 64], bf16)
        nc.vector.memset(warm[:], 0.0)
        wps = tpsum.tile([64, 64], fp32, tag="tp")
        for _ in range(40):
            nc.tensor.matmul(wps[:], warm[:, 0:64], warm[:], start=True,
                             stop=True, tile_position=(0, 0))

        ppool = ctx.enter_context(tc.tile_pool(name="patch", bufs=3))
        p2pool = ctx.enter_context(tc.tile_pool(name="patch2", bufs=3))

        def priors(k):
            # ---- priors on PE (fp32r: full-rate, near-fp32 precision) ----
            u_sb = upool.tile([P, NIN * 256], bf16, tag="u")
            v0_ps = vpsum.tile([P, 256], fp32, tag="v0ps")
            # note: u's n-axis is in issue order n = t*G + g; routing is
            # symmetric over n so the order is free.
            # all v0 matmuls first: v0 completes early so the first prep
            # chain overlaps the u fill instead of following it
            for t in range(NTAP):
                i, j = divmod(t, 3)
                # channel-major window for the K=128 v0 matmul
                patch2 = p2pool.tile([CIN, P], f32r, tag="patch2")
                nc.scalar.copy(
                    patch2[:].rearrange("c (h w) -> c h w", h=CHUNK_ROWS),
                    slab2[:, 4 * k + i: 4 * k + i + CHUNK_ROWS, j:j + W])
                nc.tensor.matmul(v0_ps[:], patch2[:],
                                 wr2[:, t * 256:(t + 1) * 256],
                                 start=(t == 0), stop=(t == NTAP - 1),
                                 tile_position=(0, 0), skip_group_check=True)
            # v_u copy issued before the u drains so it isn't queued behind
            # 18 ACT copies (prep chain would stall mid-fill otherwise)
            v_u = small.tile([P, 256], fp32, tag="vu")
            nc.scalar.copy(v_u[:], v0_ps[:])
            for t in range(NTAP):
                i, j = divmod(t, 3)
                # contiguous tap window: patch[m, (g, poh, ow)]
                patch = ppool.tile([M, G * P], f32r, tag="patch")
                nc.gpsimd.tensor_copy(
                    patch[:].rearrange("m (g h w) -> m g h w", g=G,
                                       h=CHUNK_ROWS),
                    slab[:, :, 4 * k + i: 4 * k + i + CHUNK_ROWS, j:j + W])
                rhs = wr_r[:, t * 256:(t + 1) * 256]
                for g in range(G):
                    n = t * G + g
                    lhsT = patch[:, g * P:(g + 1) * P]
                    if n % 2 == 0:
                        ups = psum.tile([P, 512], fp32, tag="ups")
                    nc.tensor.matmul(ups[:, (n % 2) * 256:(n % 2 + 1) * 256],
                                     lhsT, rhs, start=True, stop=True,
                                     tile_position=(0, 0))
                    if n % 2 == 1:
                        # drain two adjacent u blocks with one copy; during
                        # the chunk-0 fill DVE is idle, so share the load
                        dst = u_sb[:, (n - 1) * 256:(n + 1) * 256]
                        if k == 0 and (n // 2) % 2 == 0:
                            nc.vector.tensor_copy(dst, ups[:])
                        else:
                            nc.scalar.copy(dst, ups[:])

            S = small.tile([P, O], fp32, tag="S")
            return {"u_sb": u_sb, "v_u": v_u, "S": S, "vn": None}

        def prep(st):
            # vn = v_u * rsqrt(max(sum_l v_u^2, eps)); rsqrt = exp(-ln/2)
            v_u = st["v_u"]
            sq = small.tile([P, 256], fp32, tag="sq")
            nc.vector.tensor_mul(sq[:], v_u[:], v_u[:])
            w2 = small.tile([P, O], fp32, tag="w2")
            nc.vector.reduce_sum(
                w2[:], sq[:].rearrange("p (l o) -> p o l", l=L), axis=AX.X)
            nc.vector.tensor_scalar_max(w2[:], w2[:], 1e-24)
            lg = small.tile([P, O], fp32, tag="lg")
            rn = small.tile([P, O], fp32, tag="rn")
            with tc.high_priority():
                nc.scalar.activation(lg[:], w2[:], AF.Ln)
                nc.scalar.activation(rn[:], lg[:], AF.Exp, scale=-0.5)
            vn = small.tile([P, 256], bf16, tag="vn")
            nc.vector.tensor_mul(
                vn[:].rearrange("p (l o) -> p l o", l=L),
                v_u[:].rearrange("p (l o) -> p l o", l=L),
                rn[:].unsqueeze(1).broadcast_to([P, L, O]))
            st["vn"] = vn

        def main(st, it):
            u_sb, vn, S = st["u_sb"], st["vn"], st["S"]
            # one tile: z region [0:18432] + tree scratch [18432:30464]
            zt = zpool.tile([P, NIN * 256 + 12032], bf16, tag="z")
            z = zt[:, 0:NIN * 256]
            tr = zt[:, NIN * 256:]

            def add(out, a, b):
                nc.vector.tensor_add(out, a, b)

            logits = small.tile([P, NIN * O], fp32, tag="logits")
            e = small.tile([P, NIN * O], bf16, tag="e")
            NH = NIN // 2  # 36
            # n-halves pipeline: m1 -> l-tree -> exp -> m2 per half, so
            # ACT exp of half 0 overlaps DVE m2 of half 1 (and chunk-1
            # fill can start after half the u blocks land).
            for h in range(2):
                zh = z[:, h * 9216:(h + 1) * 9216]
                uh = u_sb[:, h * 9216:(h + 1) * 9216].rearrange(
                    "p (n l o) -> p n l o", n=NH, l=L)
                trh = tr[:, h * 4608:(h + 1) * 4608]
                zv = lambda ap, lw, nn=NH: ap.rearrange(
                    "p (n l o) -> p n l o", n=nn, l=lw)
                # z1 = u * bcast_n(vn)   (bf16 2x)
                nc.vector.tensor_mul(
                    zv(zh, L), uh,
                    vn[:].rearrange("p (l o) -> p l o", l=L)
                    .unsqueeze(1).broadcast_to([P, NH, L, O]))
                add(zv(trh, 8), zv(zh, L)[:, :, 0:8, :],
                    zv(zh, L)[:, :, 8:16, :])
                add(zv(zh[:, 0:2304], 4)[:, :, :, :],
                    zv(trh, 8)[:, :, 0:4, :], zv(trh, 8)[:, :, 4:8, :])
                add(zv(tr[:, 9216 + h * 1152: 9216 + (h + 1) * 1152], 2),
                    zv(zh[:, 0:2304], 4)[:, :, 0:2, :],
                    zv(zh[:, 0:2304], 4)[:, :, 2:4, :])
                lgh = logits[:, h * NH * O:(h + 1) * NH * O]
                t2 = zv(tr[:, 9216 + h * 1152: 9216 + (h + 1) * 1152], 2)
                add(lgh.rearrange("p (n o) -> p n o", n=NH),
                    t2[:, :, 0, :], t2[:, :, 1, :])
                with tc.high_priority():
                    nc.scalar.activation(e[:, h * NH * O:(h + 1) * NH * O],
                                         lgh, AF.Exp)
                # z2 = u * bcast_l(e)   (bf16 2x)
                nc.vector.tensor_mul(
                    zv(zh, L), uh,
                    e[:, h * NH * O:(h + 1) * NH * O].rearrange(
                        "p (n o) -> p n o", n=NH)
                    .unsqueeze(2).broadcast_to([P, NH, L, O]))
            if it == 2:
                nc.vector.reduce_sum(
                    S[:], e[:].rearrange("p (n o) -> p o n", n=NIN),
                    axis=AX.X)
            # v_u = sum_n z2: n-halving tree over contiguous blocks
            add(tr[:, 0:9216], z[:, 0:9216], z[:, 9216:18432])      # 36
            add(z[:, 0:4608], tr[:, 0:4608], tr[:, 4608:9216])      # 18
            add(tr[:, 9216:11520], z[:, 0:2304], z[:, 2304:4608])   # 9
            add(z[:, 0:1024],
                tr[:, 9216:10240], tr[:, 10240:11264])              # 8->4
            add(tr[:, 11520:12032], z[:, 0:512], z[:, 512:1024])    # 4->2
            add(z[:, 0:256], tr[:, 11520:11776], tr[:, 11776:12032])
            v_u = small.tile([P, 256], fp32, tag="vu")
            add(v_u[:], z[:, 0:256], tr[:, 11264:11520])  # + 9th row
            st["v_u"] = v_u

        def squash(st, k):
            # ---- squash: out = v_u * sqrt(w2) / (S^2 + w2) ----
            v_u, S = st["v_u"], st["S"]
            sq = small.tile([P, 256], fp32, tag="sq")
            nc.vector.tensor_mul(sq[:], v_u[:], v_u[:])
            w2 = small.tile([P, O], fp32, tag="w2")
            nc.vector.reduce_sum(
                w2[:], sq[:].rearrange("p (l o) -> p o l", l=L), axis=AX.X)
            nc.vector.tensor_scalar_max(w2[:], w2[:], 1e-24)
            lg = small.tile([P, O], fp32, tag="lg")
            nc.scalar.activation(lg[:], w2[:], AF.Ln)
            sw = small.tile([P, O], fp32, tag="sw")
            nc.scalar.activation(sw[:], lg[:], AF.Exp, scale=0.5)
            den = small.tile([P, O], fp32, tag="den")
            nc.vector.tensor_mul(den[:], S[:], S[:])
            nc.vector.tensor_add(den[:], den[:], w2[:])
            rden = small.tile([P, O], fp32, tag="rn")
            nc.vector.reciprocal(rden[:], den[:])
            fac = small.tile([P, O], fp32, tag="fac")
            nc.vector.tensor_mul(fac[:], sw[:], rden[:])
            # vfin [p, (o,l)] = v_u viewed (o,l) * bcast_l(fac)
            vfin = small.tile([P, 256], fp32, tag="vfin")
            nc.vector.tensor_mul(
                vfin[:].rearrange("p (o l) -> p o l", o=O),
                v_u[:].rearrange("p (l o) -> p o l", l=L),
                fac[:].unsqueeze(2).broadcast_to([P, O, L]))
            # transpose to channel-major and store
            for half in range(2):
                tp = tpsum.tile([128, 128], fp32, tag="tp")
                nc.tensor.transpose(tp[:], vfin[:, half * 128:(half + 1) * 128],
                                    ident[:])
                vT = small.tile([128, 128], fp32, tag="vT")
                nc.scalar.copy(vT[:], tp[:])
                nc.sync.dma_start(
                    out=out_d[half * 128:(half + 1) * 128,
                              4 * k:4 * k + CHUNK_ROWS, :],
                    in_=vT[:].rearrange("f (r w) -> f r w", r=CHUNK_ROWS))

        # Interleave the two chunks' routing iterations: chunk k's small
        # ACT chains (prep/exp) overlap the other chunk's DVE work. Chunk
        # 1's priors are issued under chunk 0's first iteration so its ACT
        # copies don't delay chunk 0's start.
        st0 = priors(0)
        prep(st0)
        main(st0, 0)
        prep(st0)
        st1 = priors(1)
        prep(st1)
        sts = [st0, st1]
        main(st1, 0)
        prep(st1)
        for it in range(1, 3):
            for k in range(NCHUNK):
                main(sts[k], it)
                if it < 2:
                    prep(sts[k])
                else:
                    squash(sts[k], k)
    nc.compile()
    return nc


_NC_CACHE = {}


def _get_nc():
    if "nc" not in _NC_CACHE:
        _NC_CACHE["nc"] = _build_bass()
    return _NC_CACHE["nc"]


def _shard_inputs(x, weight):
    # wr[m, (t, l, o)] = weight[o, l, m, i, j], t = i*3+j
    wr = np.ascontiguousarray(
        weight.transpose(2, 3, 4, 1, 0).reshape(M, NTAP * 256)
        .astype(np.float32))
    in_maps = []
    for core in range(NCORES):
        b = core // 4
        oh0 = (core % 4) * ROWS_PER_CORE
        xs = np.zeros((CIN, 10, 34), np.float32)
        lo, hi = oh0 - 1, oh0 + 9
        vlo, vhi = max(lo, 0), min(hi, H)
        xs[:, vlo - lo:vhi - lo, 1:33] = x[b, :, vlo:vhi, :]
        # [c=(g,m), h, w34] -> [m, g, h, w34]
        xs_m = np.ascontiguousarray(
            xs.reshape(G, M, 10, 34).transpose(1, 0, 2, 3))
        in_maps.append({"xs": xs_m, "xs2": xs, "wgt": wr})
    return in_maps


def _gather_output(results):
    out = np.zeros((B, COUT, H, W), np.float32)
    for core in range(NCORES):
        b = core // 4
        oh0 = (core % 4) * ROWS_PER_CORE
        out[b, :, oh0:oh0 + ROWS_PER_CORE, :] = results[core]["out"]
    return out


def kernel(x: np.ndarray, weight: np.ndarray) -> np.ndarray:
    from concourse.bass_utils import run_bass_kernel_spmd

    x = np.asarray(x, np.float32)
    weight = np.asarray(weight, np.float32)
    res = run_bass_kernel_spmd(_get_nc(), _shard_inputs(x, weight),
                               list(range(NCORES)))
    return _gather_output(res.results)



# revision 8
# speedup vs baseline: 1.1382x; 1.1382x over previous
"""CapsuleConv2d (k-means routing, 3 iters) Trainium2 Bass kernel.

Problem (hardcoded): x [2,128,32,32] f32, weight [16,16,16,3,3] f32
(w[o,l,m,i,j]), stride 1, pad 1, G=8 groups of M=16 in-channels,
N_in = G*KH*KW = 72 votes, O=16 out-capsules of L=16.
Output [2, 256, 32, 32] f32.

Sharding: data-parallel over (b, oh): 64 rows -> 8 cores x 8 rows.
Each core processes 2 chunks of 128 positions (4 oh-rows x 32 ow).

v2 changes over the original baseline (290us):
 - Host supplies 3 column-shifted slabs (one per kw tap j), so every
   3x3-tap window is a contiguous [*, 128] lhsT slice: the Pool/ACT
   patch-assembly copies are gone and matmuls read the slab directly.
 - The Pool engine runs the full routing chain (z1 / l-tree / z2 /
   n-tree) for votes n in [57, 72) concurrently with DVE's n in
   [0, 57): per-iteration wall time drops from ~37.6us (DVE alone at
   2x bf16) to ~31us (DVE ~30us || Pool ~30us), merged by one add.
 - u PSUM->SBUF drains are split across DVE/ACT/Pool at the head
   (chunk 0) and ride on ACT afterwards.
"""
from contextlib import ExitStack

import numpy as np

B, CIN, H, W = 2, 128, 32, 32
G, M, O, L = 8, 16, 16, 16
NTAP, NIN = 9, 72
COUT = O * L
NCORES = 8
ROWS_PER_CORE = 8  # (b, oh) rows per core
CHUNK_ROWS = 4
NCHUNK = ROWS_PER_CORE // CHUNK_ROWS
P = 128
ND = 57            # votes routed on DVE
NP = NIN - ND      # votes routed on Pool (15)
NH0, NH1 = 29, 28  # DVE n-halves for the exp pipeline


def _build_bass():
    import concourse.tile as tile
    from concourse import bacc, masks, mybir

    # The act-table pass greedily picks the first set containing each
    # function, ping-ponging exp_and_others <-> natural_log (2.7us/load).
    # Strip Exp/Ln from every set except the combined one so all our ACT
    # work (Exp, Ln, Copy, Identity) lives in a single table set.
    if not getattr(bacc, "_capsule_act_tables_patched", False):
        _orig_gat = bacc.get_activation_tables

        def _gat(arch):
            t = dict(_orig_gat(arch))
            for name, fns in t.items():
                if name != "natural_log_exp_and_others":
                    t[name] = {f for f in fns if f.name not in ("Exp", "Ln")}
            return t

        bacc.get_activation_tables = _gat
        bacc._capsule_act_tables_patched = True

    fp32 = mybir.dt.float32
    f32r = mybir.dt.float32r
    bf16 = mybir.dt.bfloat16
    AX = mybir.AxisListType
    AF = mybir.ActivationFunctionType

    nc = bacc.Bacc("TRN2", target_bir_lowering=False, debug=False)
    # j-shifted m-major slabs: xs[m, j, g, h(10), w(32)]; window for tap
    # (i, j) rows r..r+3 is contiguous 128 floats -> direct matmul lhsT.
    xs_d = nc.declare_dram_parameter("xs", [M, 3 * G * 10 * 32], fp32,
                                     isOutput=False)
    # j-shifted channel-major slabs: xs2[c=(g,m), j, h(10), w(32)]
    xs2_d = nc.declare_dram_parameter("xs2", [CIN, 3 * 10 * 32], fp32,
                                      isOutput=False)
    # host-pretransposed weights: wr[m, (tap, l, o)] = w[o, l, m, i, j]
    w_d = nc.declare_dram_parameter("wgt", [M, NTAP * 256], fp32, isOutput=False)
    out_d = nc.declare_dram_parameter("out", [COUT, ROWS_PER_CORE, W], fp32,
                                      isOutput=True)

    with tile.TileContext(nc) as tc, ExitStack() as ctx:
        const_pool = ctx.enter_context(tc.tile_pool(name="const", bufs=1))
        upool = ctx.enter_context(tc.tile_pool(name="u", bufs=2))
        zpool = ctx.enter_context(tc.tile_pool(name="z", bufs=1))
        lepool = ctx.enter_context(tc.tile_pool(name="le", bufs=1))
        small = ctx.enter_context(tc.tile_pool(name="small", bufs=2))
        psum = ctx.enter_context(tc.tile_pool(name="ps", bufs=4, space="PSUM"))
        tpsum = ctx.enter_context(tc.tile_pool(name="tps", bufs=2, space="PSUM"))
        vpsum = ctx.enter_context(tc.tile_pool(name="vps", bufs=2, space="PSUM"))

        # ---- constants (once per core) ----
        # weights replicated over g: wr2[(g,m), (t,l,o)]; wr_r = g=0 slice.
        # One cast-DMA into f32r, then log2(G) doubling SBUF->SBUF copies.
        wr2 = const_pool.tile([CIN, NTAP * 256], f32r)
        nc.gpsimd.dma_start(out=wr2[0:M, :], in_=w_d[:])
        for r in (16, 32, 64):
            nc.sync.dma_start(out=wr2[r:2 * r, :], in_=wr2[0:r, :])
        wr_r = wr2[0:M, :]

        # m-major slab, f32r via gpsimd cast-DMA; rows 0..5 first so chunk-0
        # windows are ready early.
        slab_f = const_pool.tile([M, 3 * G * 10 * 32], f32r)
        slab = slab_f[:].rearrange("m (j g h w) -> m j g h w", j=3, g=G, h=10)
        xs_v = xs_d[:].rearrange("m (j g h w) -> m j g h w", j=3, g=G, h=10)
        nc.gpsimd.dma_start(out=slab[:, :, :, 0:6, :], in_=xs_v[:, :, :, 0:6, :])
        nc.gpsimd.dma_start(out=slab[:, :, :, 6:10, :], in_=xs_v[:, :, :, 6:10, :])

        # channel-major slab for the K=128 v0 matmuls
        slab2_f = const_pool.tile([CIN, 3 * 10 * 32], f32r)
        slab2 = slab2_f[:].rearrange("c (j h w) -> c j h w", j=3, h=10)
        xs2_v = xs2_d[:].rearrange("c (j h w) -> c j h w", j=3, h=10)
        nc.gpsimd.dma_start(out=slab2[:, :, 0:6, :], in_=xs2_v[:, :, 0:6, :])
        nc.gpsimd.dma_start(out=slab2[:, :, 6:10, :], in_=xs2_v[:, :, 6:10, :])

        ident = const_pool.tile([128, 128], fp32)
        masks.make_identity(nc, ident[:])

        # PE warm-up: ~4us of back-to-back dummy matmuls during the initial
        # DMA wait releases the HAM clock throttle before the real matmuls.
        warm = const_pool.tile([128, 64], bf16)
        nc.vector.memset(warm[:], 0.0)
        wps = tpsum.tile([64, 64], fp32, tag="tp")
        for _ in range(40):
            nc.tensor.matmul(wps[:], warm[:, 0:64], warm[:], start=True,
                             stop=True, tile_position=(0, 0))

        def priors(k):
            # ---- priors on PE (fp32r: full-rate, near-fp32 precision) ----
            u_sb = upool.tile([P, NIN * 256], bf16, tag="u")
            v0_ps = vpsum.tile([P, 256], fp32, tag="v0ps")
            # all v0 matmuls first: v0 completes early so the first prep
            # chain overlaps the u fill instead of following it
            for t in range(NTAP):
                i, j = divmod(t, 3)
                lhsT2 = slab2[:, j, 4 * k + i: 4 * k + i + CHUNK_ROWS, :]
                nc.tensor.matmul(v0_ps[:], lhsT2,
                                 wr2[:, t * 256:(t + 1) * 256],
                                 start=(t == 0), stop=(t == NTAP - 1),
                                 tile_position=(0, 0), skip_group_check=True)
            # v_u copy issued before the u drains so it isn't queued behind
            # the drain copies (prep chain would stall mid-fill otherwise)
            v_u = small.tile([P, 256], fp32, tag="vu")
            nc.scalar.copy(v_u[:], v0_ps[:])
            for t in range(NTAP):
                i, j = divmod(t, 3)
                rhs = wr_r[:, t * 256:(t + 1) * 256]
                for g in range(G):
                    n = t * G + g
                    lhsT = slab[:, j, g, 4 * k + i: 4 * k + i + CHUNK_ROWS, :]
                    if n % 2 == 0:
                        ups = psum.tile([P, 512], fp32, tag="ups")
                    nc.tensor.matmul(ups[:, (n % 2) * 256:(n % 2 + 1) * 256],
                                     lhsT, rhs, start=True, stop=True,
                                     tile_position=(0, 0))
                    if n % 2 == 1:
                        # drain two adjacent u blocks with one copy; chunk 0
                        # splits the drains across DVE/ACT/Pool (all three
                        # are idle at the head), chunk 1 rides on ACT whose
                        # steady-state slack covers it.
                        dst = u_sb[:, (n - 1) * 256:(n + 1) * 256]
                        if k == 0 and (n // 2) % 2 == 0:
                            nc.vector.tensor_copy(dst, ups[:])
                        else:
                            nc.scalar.copy(dst, ups[:])

            S = small.tile([P, O], fp32, tag="S")
            return {"u_sb": u_sb, "v_u": v_u, "S": S, "vn": None}

        def prep(st):
            # vn = v_u * rsqrt(max(sum_l v_u^2, eps)); rsqrt = exp(-ln/2)
            v_u = st["v_u"]
            sq = small.tile([P, 256], fp32, tag="sq")
            nc.vector.tensor_mul(sq[:], v_u[:], v_u[:])
            w2 = small.tile([P, O], fp32, tag="w2")
            nc.vector.reduce_sum(
                w2[:], sq[:].rearrange("p (l o) -> p o l", l=L), axis=AX.X)
            nc.vector.tensor_scalar_max(w2[:], w2[:], 1e-24)
            lg = small.tile([P, O], fp32, tag="lg")
            rn = small.tile([P, O], fp32, tag="rn")
            with tc.high_priority():
                nc.scalar.activation(lg[:], w2[:], AF.Ln)
                nc.scalar.activation(rn[:], lg[:], AF.Exp, scale=-0.5)
            vn = small.tile([P, 256], bf16, tag="vn")
            nc.vector.tensor_mul(
                vn[:].rearrange("p (l o) -> p l o", l=L),
                v_u[:].rearrange("p (l o) -> p l o", l=L),
                rn[:].unsqueeze(1).broadcast_to([P, L, O]))
            st["vn"] = vn

        def main(st, it):
            u_sb, vn, S = st["u_sb"], st["vn"], st["S"]
            vn_lo = vn[:].rearrange("p (l o) -> p l o", l=L)
            # DVE z region [0 : ND*256] + tree scratch (tr: max(l-tree h0
            # NH0*160, n-tree L1 28*256) = 7424; tr2: l-tree h1 NH1*160)
            TRW = NH0 * 256  # 7424
            zt = zpool.tile([P, ND * 256 + TRW + NH1 * 160], bf16, tag="z")
            z = zt[:, 0:ND * 256]
            tr = zt[:, ND * 256: ND * 256 + TRW]
            tr2 = zt[:, ND * 256 + TRW:]
            # Pool z region (ptr covers the l-tree's NP*160 = 2400 elems)
            pzt = zpool.tile([P, NP * 256 + 2400 + 512], bf16, tag="pz")
            pz = pzt[:, 0:NP * 256]
            ptr = pzt[:, NP * 256: NP * 256 + 2400]
            ptr2 = pzt[:, NP * 256 + 2400:]

            # logits/e lifetime is within this main call (mains execute
            # in-order per engine), so single-buffered is safe.
            logits = lepool.tile([P, NIN * O], fp32, tag="logits")
            e = lepool.tile([P, NIN * O], bf16, tag="e")

            def zv(ap, lw, nn):
                return ap.rearrange("p (n l o) -> p n l o", n=nn, l=lw)

            def half(eng, zh, uh, trh, lgh, nh):
                # z1 = u * bcast_n(vn); l-tree; logits slice
                eng.tensor_mul(
                    zv(zh, L, nh), uh,
                    vn_lo.unsqueeze(1).broadcast_to([P, nh, L, O]))
                eng.tensor_add(zv(trh[:, 0:nh * 128], 8, nh),
                               zv(zh, L, nh)[:, :, 0:8, :],
                               zv(zh, L, nh)[:, :, 8:16, :])
                eng.tensor_add(zv(zh[:, 0:nh * 64], 4, nh),
                               zv(trh[:, 0:nh * 128], 8, nh)[:, :, 0:4, :],
                               zv(trh[:, 0:nh * 128], 8, nh)[:, :, 4:8, :])
                eng.tensor_add(zv(trh[:, nh * 128:nh * 160], 2, nh),
                               zv(zh[:, 0:nh * 64], 4, nh)[:, :, 0:2, :],
                               zv(zh[:, 0:nh * 64], 4, nh)[:, :, 2:4, :])
                t2 = zv(trh[:, nh * 128:nh * 160], 2, nh)
                eng.tensor_add(lgh.rearrange("p (n o) -> p n o", n=nh),
                               t2[:, :, 0, :], t2[:, :, 1, :])

            def z2(eng, zh, uh, eh, nh):
                eng.tensor_mul(
                    zv(zh, L, nh), uh,
                    eh.rearrange("p (n o) -> p n o", n=nh)
                    .unsqueeze(2).broadcast_to([P, nh, L, O]))

            # ---- DVE slice: two n-halves pipelined through the ACT exp ----
            offs = [(0, NH0), (NH0, NH1)]
            for (n0, nh) in offs:
                zh = z[:, n0 * 256:(n0 + nh) * 256]
                uh = zv(u_sb[:, n0 * 256:(n0 + nh) * 256], L, nh)
                trh = tr if n0 == 0 else tr2
                lgh = logits[:, n0 * O:(n0 + nh) * O]
                half(nc.vector, zh, uh, trh, lgh, nh)
                with tc.high_priority():
                    nc.scalar.activation(e[:, n0 * O:(n0 + nh) * O], lgh,
                                         AF.Exp)
                z2(nc.vector, zh, uh, e[:, n0 * O:(n0 + nh) * O], nh)

            # ---- Pool slice: same chain for n in [ND, NIN) ----
            pzh = pz
            puh = zv(u_sb[:, ND * 256:], L, NP)
            plg = logits[:, ND * O:]
            half(nc.gpsimd, pzh, puh, ptr, plg, NP)
            with tc.high_priority():
                nc.scalar.activation(e[:, ND * O:], plg, AF.Exp)
            z2(nc.gpsimd, pzh, puh, e[:, ND * O:], NP)

            if it == 2:
                nc.vector.reduce_sum(
                    S[:], e[:].rearrange("p (n o) -> p o n", n=NIN),
                    axis=AX.X)

            def add(eng, out, a, b):
                eng.tensor_add(out, a, b)

            # ---- DVE n-tree: 57 -> 28(+1) -> 14(+1c) ... leftovers merged
            # at the end. Block row = 256 elems.
            A = nc.vector
            add(A, tr[:, 0:28 * 256], z[:, 0:28 * 256], z[:, 28 * 256:56 * 256])
            # leftover: z row 56
            add(A, z[:, 0:14 * 256], tr[:, 0:14 * 256], tr[:, 14 * 256:28 * 256])
            add(A, tr[:, 0:7 * 256], z[:, 0:7 * 256], z[:, 7 * 256:14 * 256])
            add(A, z[:, 0:3 * 256], tr[:, 0:3 * 256], tr[:, 3 * 256:6 * 256])
            # leftover: tr row 6
            add(A, tr2[:, 0:256], z[:, 0:256], z[:, 256:512])
            # rows left: tr2[0], z row2, tr row6, z row56
            add(A, z[:, 256:512], z[:, 2 * 256:3 * 256], z[:, 56 * 256:57 * 256])
            add(A, z[:, 0:256], tr2[:, 0:256], tr[:, 6 * 256:7 * 256])
            v_d = small.tile([P, 256], fp32, tag="vd")
            add(A, v_d[:], z[:, 0:256], z[:, 256:512])

            # ---- Pool n-tree: 15 -> 7(+1) -> 3(+1c) -> 1(+1c)
            Pp = nc.gpsimd
            add(Pp, ptr[:, 0:7 * 256], pz[:, 0:7 * 256], pz[:, 7 * 256:14 * 256])
            # leftover pz row 14
            add(Pp, pz[:, 0:3 * 256], ptr[:, 0:3 * 256], ptr[:, 3 * 256:6 * 256])
            # leftover ptr row 6
            add(Pp, ptr2[:, 0:256], pz[:, 0:256], pz[:, 256:512])
            add(Pp, pz[:, 0:256], pz[:, 2 * 256:3 * 256], pz[:, 14 * 256:15 * 256])
            add(Pp, ptr2[:, 256:512], ptr2[:, 0:256], ptr[:, 6 * 256:7 * 256])
            v_p = small.tile([P, 256], fp32, tag="vp")
            add(Pp, v_p[:], pz[:, 0:256], ptr2[:, 256:512])

            # merge
            v_u = small.tile([P, 256], fp32, tag="vu")
            nc.vector.tensor_add(v_u[:], v_d[:], v_p[:])
            st["v_u"] = v_u

        def squash(st, k):
            # ---- squash: out = v_u * sqrt(w2) / (S^2 + w2) ----
            v_u, S = st["v_u"], st["S"]
            sq = small.tile([P, 256], fp32, tag="sq")
            nc.vector.tensor_mul(sq[:], v_u[:], v_u[:])
            w2 = small.tile([P, O], fp32, tag="w2")
            nc.vector.reduce_sum(
                w2[:], sq[:].rearrange("p (l o) -> p o l", l=L), axis=AX.X)
            nc.vector.tensor_scalar_max(w2[:], w2[:], 1e-24)
            lg = small.tile([P, O], fp32, tag="lg")
            nc.scalar.activation(lg[:], w2[:], AF.Ln)
            sw = small.tile([P, O], fp32, tag="sw")
            nc.scalar.activation(sw[:], lg[:], AF.Exp, scale=0.5)
            den = small.tile([P, O], fp32, tag="den")
            nc.vector.tensor_mul(den[:], S[:], S[:])
            nc.vector.tensor_add(den[:], den[:], w2[:])
            rden = small.tile([P, O], fp32, tag="rn")
            nc.vector.reciprocal(rden[:], den[:])
            fac = small.tile([P, O], fp32, tag="fac")
            nc.vector.tensor_mul(fac[:], sw[:], rden[:])
            # vfin [p, (o,l)] = v_u viewed (o,l) * bcast_l(fac)
            vfin = small.tile([P, 256], fp32, tag="vfin")
            nc.vector.tensor_mul(
                vfin[:].rearrange("p (o l) -> p o l", o=O),
                v_u[:].rearrange("p (l o) -> p o l", l=L),
                fac[:].unsqueeze(2).broadcast_to([P, O, L]))
            # transpose to channel-major and store
            for half_i in range(2):
                tp = tpsum.tile([128, 128], fp32, tag="tp")
                nc.tensor.transpose(tp[:],
                                    vfin[:, half_i * 128:(half_i + 1) * 128],
                                    ident[:])
                vT = small.tile([128, 128], fp32, tag="vT")
                nc.scalar.copy(vT[:], tp[:])
                nc.sync.dma_start(
                    out=out_d[half_i * 128:(half_i + 1) * 128,
                              4 * k:4 * k + CHUNK_ROWS, :],
                    in_=vT[:].rearrange("f (r w) -> f r w", r=CHUNK_ROWS))

        # Interleave the two chunks' routing iterations: chunk k's small
        # ACT chains (prep/exp) overlap the other chunk's DVE/Pool work.
        st0 = priors(0)
        prep(st0)
        main(st0, 0)
        prep(st0)
        st1 = priors(1)
        prep(st1)
        sts = [st0, st1]
        main(st1, 0)
        prep(st1)
        for it in range(1, 3):
            for k in range(NCHUNK):
                main(sts[k], it)
                if it < 2:
                    prep(sts[k])
                else:
                    squash(sts[k], k)
    nc.compile()
    return nc


_NC_CACHE = {}


def _get_nc():
    if "nc" not in _NC_CACHE:
        _NC_CACHE["nc"] = _build_bass()
    return _NC_CACHE["nc"]


def _shard_inputs(x, weight):
    # wr[m, (t, l, o)] = weight[o, l, m, i, j], t = i*3+j
    wr = np.ascontiguousarray(
        weight.transpose(2, 3, 4, 1, 0).reshape(M, NTAP * 256)
        .astype(np.float32))
    in_maps = []
    for core in range(NCORES):
        b = core // 4
        oh0 = (core % 4) * ROWS_PER_CORE
        xs = np.zeros((CIN, 10, 34), np.float32)
        lo, hi = oh0 - 1, oh0 + 9
        vlo, vhi = max(lo, 0), min(hi, H)
        xs[:, vlo - lo:vhi - lo, 1:33] = x[b, :, vlo:vhi, :]
        # j-shifted channel-major: xs2j[c, j, h, 32]
        xs2j = np.stack([xs[:, :, j:j + 32] for j in range(3)], axis=1)
        xs2j = np.ascontiguousarray(xs2j.reshape(CIN, 3 * 10 * 32))
        # j-shifted m-major: xsj[m, j, g, h, 32]
        xs_m = xs.reshape(G, M, 10, 34)
        xsj = np.stack([xs_m[:, :, :, j:j + 32] for j in range(3)], axis=2)
        # [g, m, j, h, w] -> [m, j, g, h, w]
        xsj = np.ascontiguousarray(
            xsj.transpose(1, 2, 0, 3, 4).reshape(M, 3 * G * 10 * 32))
        in_maps.append({"xs": xsj, "xs2": xs2j, "wgt": wr})
    return in_maps


def _gather_output(results):
    out = np.zeros((B, COUT, H, W), np.float32)
    for core in range(NCORES):
        b = core // 4
        oh0 = (core % 4) * ROWS_PER_CORE
        out[b, :, oh0:oh0 + ROWS_PER_CORE, :] = results[core]["out"]
    return out


def kernel(x: np.ndarray, weight: np.ndarray) -> np.ndarray:
    from concourse.bass_utils import run_bass_kernel_spmd

    x = np.asarray(x, np.float32)
    weight = np.asarray(weight, np.float32)
    res = run_bass_kernel_spmd(_get_nc(), _shard_inputs(x, weight),
                               list(range(NCORES)))
    return _gather_output(res.results)


# revision 14
# speedup vs baseline: 1.1481x; 1.0087x over previous
"""CapsuleConv2d (k-means routing, 3 iters) Trainium2 Bass kernel.

Problem (hardcoded): x [2,128,32,32] f32, weight [16,16,16,3,3] f32
(w[o,l,m,i,j]), stride 1, pad 1, G=8 groups of M=16 in-channels,
N_in = G*KH*KW = 72 votes, O=16 out-capsules of L=16.
Output [2, 256, 32, 32] f32.

Sharding: data-parallel over (b, oh): 64 rows -> 8 cores x 8 rows.
Each core processes 2 chunks of 128 positions (4 oh-rows x 32 ow).

v2 changes over the original baseline (290us):
 - Host supplies 3 column-shifted slabs (one per kw tap j), so every
   3x3-tap window is a contiguous [*, 128] lhsT slice: the Pool/ACT
   patch-assembly copies are gone and matmuls read the slab directly.
 - The Pool engine runs the full routing chain (z1 / l-tree / z2 /
   n-tree) for votes n in [57, 72) concurrently with DVE's n in
   [0, 57): per-iteration wall time drops from ~37.6us (DVE alone at
   2x bf16) to ~31us (DVE ~30us || Pool ~30us), merged by one add.
 - u PSUM->SBUF drains are split across DVE/ACT/Pool at the head
   (chunk 0) and ride on ACT afterwards.
"""
from contextlib import ExitStack

import numpy as np

B, CIN, H, W = 2, 128, 32, 32
G, M, O, L = 8, 16, 16, 16
NTAP, NIN = 9, 72
COUT = O * L
NCORES = 8
ROWS_PER_CORE = 8  # (b, oh) rows per core
CHUNK_ROWS = 4
NCHUNK = ROWS_PER_CORE // CHUNK_ROWS
P = 128
ND = 57            # votes routed on DVE
NP = NIN - ND      # votes routed on Pool (15)
NH0, NH1 = 29, 28  # DVE n-halves for the exp pipeline


def _build_bass():
    import concourse.tile as tile
    from concourse import bacc, masks, mybir

    # The act-table pass greedily picks the first set containing each
    # function, ping-ponging exp_and_others <-> natural_log (2.7us/load).
    # Strip Exp/Ln from every set except the combined one so all our ACT
    # work (Exp, Ln, Copy, Identity) lives in a single table set.
    if not getattr(bacc, "_capsule_act_tables_patched", False):
        _orig_gat = bacc.get_activation_tables

        def _gat(arch):
            t = dict(_orig_gat(arch))
            for name, fns in t.items():
                if name != "natural_log_exp_and_others":
                    t[name] = {f for f in fns if f.name not in ("Exp", "Ln")}
            return t

        bacc.get_activation_tables = _gat
        bacc._capsule_act_tables_patched = True

    fp32 = mybir.dt.float32
    f32r = mybir.dt.float32r
    bf16 = mybir.dt.bfloat16
    AX = mybir.AxisListType
    AF = mybir.ActivationFunctionType

    nc = bacc.Bacc("TRN2", target_bir_lowering=False, debug=False)
    # f32r params: same bits as fp32, but lets every DMA queue load them
    # without the gpsimd-only cast path.
    # j-shifted m-major slabs: xs[m, j, g, h(10), w(32)]; window for tap
    # (i, j) rows r..r+3 is contiguous 128 floats -> direct matmul lhsT.
    xs_d = nc.declare_dram_parameter("xs", [M, 3 * G * 10 * 32], f32r,
                                     isOutput=False)
    # j-shifted channel-major slabs: xs2[c=(g,m), j, h(10), w(32)]
    xs2_d = nc.declare_dram_parameter("xs2", [CIN, 3 * 10 * 32], f32r,
                                      isOutput=False)
    # host-pretransposed weights: wr[m, (tap, l, o)] = w[o, l, m, i, j]
    w_d = nc.declare_dram_parameter("wgt", [M, NTAP * 256], f32r,
                                    isOutput=False)
    out_d = nc.declare_dram_parameter("out", [COUT, ROWS_PER_CORE, W], fp32,
                                      isOutput=True)

    with tile.TileContext(nc) as tc, ExitStack() as ctx:
        const_pool = ctx.enter_context(tc.tile_pool(name="const", bufs=1))
        upool = ctx.enter_context(tc.tile_pool(name="u", bufs=2))
        zpool = ctx.enter_context(tc.tile_pool(name="z", bufs=1))
        lepool = ctx.enter_context(tc.tile_pool(name="le", bufs=1))
        small = ctx.enter_context(tc.tile_pool(name="small", bufs=2))
        psum = ctx.enter_context(tc.tile_pool(name="ps", bufs=2, space="PSUM"))
        tpsum = ctx.enter_context(tc.tile_pool(name="tps", bufs=2, space="PSUM"))
        vpsum = ctx.enter_context(tc.tile_pool(name="vps", bufs=2, space="PSUM"))

        # ---- constants (once per core) ----
        # weights replicated over g: wr2[(g,m), (t,l,o)]; wr_r = g=0 slice.
        # One DMA, then log2(G) doubling SBUF->SBUF copies spread over the
        # SP/ACT queues (d2 on ACT so it overlaps d-independent SP work).
        wr2 = const_pool.tile([CIN, NTAP * 256], f32r)
        nc.sync.dma_start(out=wr2[0:M, :], in_=w_d[:])
        nc.sync.dma_start(out=wr2[16:32, :], in_=wr2[0:16, :])
        nc.scalar.dma_start(out=wr2[32:64, :], in_=wr2[0:32, :])
        wr_r = wr2[0:M, :]

        # m-major slab; rows 0..5 first so chunk-0 windows are ready early.
        slab_f = const_pool.tile([M, 3 * G * 10 * 32], f32r)
        slab = slab_f[:].rearrange("m (j g h w) -> m j g h w", j=3, g=G, h=10)
        xs_v = xs_d[:].rearrange("m (j g h w) -> m j g h w", j=3, g=G, h=10)
        nc.sync.dma_start(out=slab[:, :, :, 0:6, :], in_=xs_v[:, :, :, 0:6, :])
        nc.sync.dma_start(out=wr2[64:128, :], in_=wr2[0:64, :])
        nc.sync.dma_start(out=slab[:, :, :, 6:10, :], in_=xs_v[:, :, :, 6:10, :])

        # channel-major slab for the K=128 v0 matmuls
        slab2_f = const_pool.tile([CIN, 3 * 10 * 32], f32r)
        slab2 = slab2_f[:].rearrange("c (j h w) -> c j h w", j=3, h=10)
        xs2_v = xs2_d[:].rearrange("c (j h w) -> c j h w", j=3, h=10)
        nc.scalar.dma_start(out=slab2[:, :, 0:6, :], in_=xs2_v[:, :, 0:6, :])
        nc.scalar.dma_start(out=slab2[:, :, 6:10, :], in_=xs2_v[:, :, 6:10, :])

        ident = const_pool.tile([128, 128], fp32)
        masks.make_identity(nc, ident[:])

        # PE warm-up: ~4us of back-to-back dummy matmuls during the initial
        # DMA wait releases the HAM clock throttle before the real matmuls.
        warm = const_pool.tile([128, 64], bf16)
        nc.vector.memset(warm[:], 0.0)
        wps = tpsum.tile([64, 64], fp32, tag="tp")
        for _ in range(40):
            nc.tensor.matmul(wps[:], warm[:, 0:64], warm[:], start=True,
                             stop=True, tile_position=(0, 0))

        def priors_v0(k):
            # ---- v0 = sum_n u via K=128 matmuls; finishes early so prep
            # (hoisted before the u fill) can compute vn immediately.
            v0_ps = vpsum.tile([P, 256], fp32, tag="v0ps")
            for t in range(NTAP):
                i, j = divmod(t, 3)
                lhsT2 = slab2[:, j, 4 * k + i: 4 * k + i + CHUNK_ROWS, :]
                nc.tensor.matmul(v0_ps[:], lhsT2,
                                 wr2[:, t * 256:(t + 1) * 256],
                                 start=(t == 0), stop=(t == NTAP - 1),
                                 tile_position=(0, 0), skip_group_check=True)
            v_u = small.tile([P, 256], fp32, tag="vu")
            nc.scalar.copy(v_u[:], v0_ps[:])
            S = small.tile([P, O], fp32, tag="S")
            return {"v_u": v_u, "S": S, "vn": None, "k": k}

        # u-slot drain groups: Pool's slice [ND, NIN) fills and drains
        # first so the Pool routing chain starts on time, then DVE's h0 and
        # h1. 'D'/'A' pick the drain engine (chunk 0 splits DVE/ACT while
        # DVE is otherwise idle; chunk 1 rides entirely on ACT slack).
        GROUPS = {
            0: [(57, 61, 'A'), (61, 65, 'A'), (65, 69, 'A'), (69, 72, 'A'),
                (0, 4, 'D'), (4, 8, 'A'), (8, 12, 'D'), (12, 16, 'A'),
                (16, 20, 'D'), (20, 24, 'A'), (24, 28, 'D'), (28, 29, 'A'),
                (29, 33, 'A'), (33, 37, 'A'), (37, 41, 'A'), (41, 45, 'A'),
                (45, 49, 'A'), (49, 53, 'A'), (53, 57, 'A')],
            1: [(57, 61, 'A'), (61, 65, 'A'), (65, 69, 'A'), (69, 72, 'A'),
                (0, 4, 'A'), (4, 8, 'A'), (8, 12, 'A'), (12, 16, 'A'),
                (16, 20, 'A'), (20, 24, 'A'), (24, 28, 'A'), (28, 29, 'A'),
                (29, 33, 'A'), (33, 37, 'A'), (37, 41, 'A'), (41, 45, 'A'),
                (45, 49, 'A'), (49, 53, 'A'), (53, 57, 'A')],
        }

        def priors_u(st):
            # ---- priors on PE (fp32r: full-rate, near-fp32 precision).
            # Slot s holds vote (t, g) = divmod(s, 8); routing is symmetric
            # over votes so the assignment is free.
            k = st["k"]
            u_sb = upool.tile([P, NIN * 256], bf16, tag="u")
            for (s0, s1, ec) in GROUPS[k]:
                ups = psum.tile([P, 1024], fp32, tag="ups")
                for s in range(s0, s1):
                    t, g = divmod(s, 8)
                    i, j = divmod(t, 3)
                    lhsT = slab[:, j, g, 4 * k + i: 4 * k + i + CHUNK_ROWS, :]
                    nc.tensor.matmul(
                        ups[:, (s - s0) * 256:(s - s0 + 1) * 256],
                        lhsT, wr_r[:, t * 256:(t + 1) * 256],
                        start=True, stop=True, tile_position=(0, 0))
                dst = u_sb[:, s0 * 256:s1 * 256]
                src = ups[:, 0:(s1 - s0) * 256]
                if ec == 'D':
                    nc.vector.tensor_copy(dst, src)
                else:
                    nc.scalar.copy(dst, src)
            st["u_sb"] = u_sb

        def prep(st):
            # vn = v_u * rsqrt(max(sum_l v_u^2, eps)); rsqrt = exp(-ln/2)
            v_u = st["v_u"]
            sq = small.tile([P, 256], fp32, tag="sq")
            nc.vector.tensor_mul(sq[:], v_u[:], v_u[:])
            w2 = small.tile([P, O], fp32, tag="w2")
            nc.vector.reduce_sum(
                w2[:], sq[:].rearrange("p (l o) -> p o l", l=L), axis=AX.X)
            nc.vector.tensor_scalar_max(w2[:], w2[:], 1e-24)
            lg = small.tile([P, O], fp32, tag="lg")
            rn = small.tile([P, O], fp32, tag="rn")
            with tc.high_priority():
                nc.scalar.activation(lg[:], w2[:], AF.Ln)
                nc.scalar.activation(rn[:], lg[:], AF.Exp, scale=-0.5)
            vn = small.tile([P, 256], bf16, tag="vn")
            nc.vector.tensor_mul(
                vn[:].rearrange("p (l o) -> p l o", l=L),
                v_u[:].rearrange("p (l o) -> p l o", l=L),
                rn[:].unsqueeze(1).broadcast_to([P, L, O]))
            st["vn"] = vn

        def main(st, it):
            u_sb, vn, S = st["u_sb"], st["vn"], st["S"]
            vn_lo = vn[:].rearrange("p (l o) -> p l o", l=L)
            # DVE z region [0 : ND*256] + tree scratch (tr: max(l-tree h0
            # NH0*160, n-tree L1 28*256) = 7424; tr2: l-tree h1 NH1*160)
            TRW = NH0 * 256  # 7424
            zt = zpool.tile([P, ND * 256 + TRW + NH1 * 160], bf16, tag="z")
            z = zt[:, 0:ND * 256]
            tr = zt[:, ND * 256: ND * 256 + TRW]
            tr2 = zt[:, ND * 256 + TRW:]
            # Pool z region (ptr covers the l-tree's NP*160 = 2400 elems)
            pzt = zpool.tile([P, NP * 256 + 2400 + 512], bf16, tag="pz")
            pz = pzt[:, 0:NP * 256]
            ptr = pzt[:, NP * 256: NP * 256 + 2400]
            ptr2 = pzt[:, NP * 256 + 2400:]

            # logits/e lifetime is within this main call (mains execute
            # in-order per engine), so single-buffered is safe.
            logits = lepool.tile([P, NIN * O], fp32, tag="logits")
            e = lepool.tile([P, NIN * O], bf16, tag="e")

            def zv(ap, lw, nn):
                return ap.rearrange("p (n l o) -> p n l o", n=nn, l=lw)

            def half(eng, zh, uh, trh, lgh, nh):
                # z1 = u * bcast_n(vn); l-tree; logits slice
                eng.tensor_mul(
                    zv(zh, L, nh), uh,
                    vn_lo.unsqueeze(1).broadcast_to([P, nh, L, O]))
                eng.tensor_add(zv(trh[:, 0:nh * 128], 8, nh),
                               zv(zh, L, nh)[:, :, 0:8, :],
                               zv(zh, L, nh)[:, :, 8:16, :])
                eng.tensor_add(zv(zh[:, 0:nh * 64], 4, nh),
                               zv(trh[:, 0:nh * 128], 8, nh)[:, :, 0:4, :],
                               zv(trh[:, 0:nh * 128], 8, nh)[:, :, 4:8, :])
                eng.tensor_add(zv(trh[:, nh * 128:nh * 160], 2, nh),
                               zv(zh[:, 0:nh * 64], 4, nh)[:, :, 0:2, :],
                               zv(zh[:, 0:nh * 64], 4, nh)[:, :, 2:4, :])
                t2 = zv(trh[:, nh * 128:nh * 160], 2, nh)
                eng.tensor_add(lgh.rearrange("p (n o) -> p n o", n=nh),
                               t2[:, :, 0, :], t2[:, :, 1, :])

            def z2(eng, zh, uh, eh, nh):
                eng.tensor_mul(
                    zv(zh, L, nh), uh,
                    eh.rearrange("p (n o) -> p n o", n=nh)
                    .unsqueeze(2).broadcast_to([P, nh, L, O]))

            # ---- DVE slice: two n-halves pipelined through the ACT exp ----
            offs = [(0, NH0), (NH0, NH1)]
            for (n0, nh) in offs:
                zh = z[:, n0 * 256:(n0 + nh) * 256]
                uh = zv(u_sb[:, n0 * 256:(n0 + nh) * 256], L, nh)
                trh = tr if n0 == 0 else tr2
                lgh = logits[:, n0 * O:(n0 + nh) * O]
                half(nc.vector, zh, uh, trh, lgh, nh)
                with tc.high_priority():
                    nc.scalar.activation(e[:, n0 * O:(n0 + nh) * O], lgh,
                                         AF.Exp)
                z2(nc.vector, zh, uh, e[:, n0 * O:(n0 + nh) * O], nh)

            # ---- Pool slice: same chain for n in [ND, NIN) ----
            pzh = pz
            puh = zv(u_sb[:, ND * 256:], L, NP)
            plg = logits[:, ND * O:]
            half(nc.gpsimd, pzh, puh, ptr, plg, NP)
            with tc.high_priority():
                nc.scalar.activation(e[:, ND * O:], plg, AF.Exp)
            z2(nc.gpsimd, pzh, puh, e[:, ND * O:], NP)

            if it == 2:
                nc.vector.reduce_sum(
                    S[:], e[:].rearrange("p (n o) -> p o n", n=NIN),
                    axis=AX.X)

            def add(eng, out, a, b):
                eng.tensor_add(out, a, b)

            # ---- DVE n-tree: 57 -> 28(+1) -> 14(+1c) ... leftovers merged
            # at the end. Block row = 256 elems.
            A = nc.vector
            add(A, tr[:, 0:28 * 256], z[:, 0:28 * 256], z[:, 28 * 256:56 * 256])
            # leftover: z row 56
            add(A, z[:, 0:14 * 256], tr[:, 0:14 * 256], tr[:, 14 * 256:28 * 256])
            add(A, tr[:, 0:7 * 256], z[:, 0:7 * 256], z[:, 7 * 256:14 * 256])
            add(A, z[:, 0:3 * 256], tr[:, 0:3 * 256], tr[:, 3 * 256:6 * 256])
            # leftover: tr row 6
            add(A, tr2[:, 0:256], z[:, 0:256], z[:, 256:512])
            # rows left: tr2[0], z row2, tr row6, z row56
            add(A, z[:, 256:512], z[:, 2 * 256:3 * 256], z[:, 56 * 256:57 * 256])
            add(A, z[:, 0:256], tr2[:, 0:256], tr[:, 6 * 256:7 * 256])
            v_d = small.tile([P, 256], fp32, tag="vd")
            add(A, v_d[:], z[:, 0:256], z[:, 256:512])

            # ---- Pool n-tree: 15 -> 7(+1) -> 3(+1c) -> 1(+1c)
            Pp = nc.gpsimd
            add(Pp, ptr[:, 0:7 * 256], pz[:, 0:7 * 256], pz[:, 7 * 256:14 * 256])
            # leftover pz row 14
            add(Pp, pz[:, 0:3 * 256], ptr[:, 0:3 * 256], ptr[:, 3 * 256:6 * 256])
            # leftover ptr row 6
            add(Pp, ptr2[:, 0:256], pz[:, 0:256], pz[:, 256:512])
            add(Pp, pz[:, 0:256], pz[:, 2 * 256:3 * 256], pz[:, 14 * 256:15 * 256])
            add(Pp, ptr2[:, 256:512], ptr2[:, 0:256], ptr[:, 6 * 256:7 * 256])
            v_p = small.tile([P, 256], fp32, tag="vp")
            add(Pp, v_p[:], pz[:, 0:256], ptr2[:, 256:512])

            # merge
            v_u = small.tile([P, 256], fp32, tag="vu")
            nc.vector.tensor_add(v_u[:], v_d[:], v_p[:])
            st["v_u"] = v_u

        def squash(st, k):
            # ---- squash: out = v_u * sqrt(w2) / (S^2 + w2) ----
            v_u, S = st["v_u"], st["S"]
            sq = small.tile([P, 256], fp32, tag="sq")
            nc.vector.tensor_mul(sq[:], v_u[:], v_u[:])
            w2 = small.tile([P, O], fp32, tag="w2")
            nc.vector.reduce_sum(
                w2[:], sq[:].rearrange("p (l o) -> p o l", l=L), axis=AX.X)
            nc.vector.tensor_scalar_max(w2[:], w2[:], 1e-24)
            lg = small.tile([P, O], fp32, tag="lg")
            nc.scalar.activation(lg[:], w2[:], AF.Ln)
            sw = small.tile([P, O], fp32, tag="sw")
            nc.scalar.activation(sw[:], lg[:], AF.Exp, scale=0.5)
            den = small.tile([P, O], fp32, tag="den")
            nc.vector.tensor_mul(den[:], S[:], S[:])
            nc.vector.tensor_add(den[:], den[:], w2[:])
            rden = small.tile([P, O], fp32, tag="rn")
            nc.vector.reciprocal(rden[:], den[:])
            fac = small.tile([P, O], fp32, tag="fac")
            nc.vector.tensor_mul(fac[:], sw[:], rden[:])
            # vfin [p, (o,l)] = v_u viewed (o,l) * bcast_l(fac)
            vfin = small.tile([P, 256], fp32, tag="vfin")
            nc.vector.tensor_mul(
                vfin[:].rearrange("p (o l) -> p o l", o=O),
                v_u[:].rearrange("p (l o) -> p o l", l=L),
                fac[:].unsqueeze(2).broadcast_to([P, O, L]))
            # transpose to channel-major and store
            for half_i in range(2):
                tp = tpsum.tile([128, 128], fp32, tag="tp")
                nc.tensor.transpose(tp[:],
                                    vfin[:, half_i * 128:(half_i + 1) * 128],
                                    ident[:])
                vT = small.tile([128, 128], fp32, tag="vT")
                nc.scalar.copy(vT[:], tp[:])
                nc.sync.dma_start(
                    out=out_d[half_i * 128:(half_i + 1) * 128,
                              4 * k:4 * k + CHUNK_ROWS, :],
                    in_=vT[:].rearrange("f (r w) -> f r w", r=CHUNK_ROWS))

        # Interleave the two chunks' routing iterations: chunk k's small
        # ACT chains (prep/exp) overlap the other chunk's DVE/Pool work.
        st0 = priors_v0(0)
        prep(st0)
        priors_u(st0)
        main(st0, 0)
        prep(st0)
        st1 = priors_v0(1)
        prep(st1)
        priors_u(st1)
        sts = [st0, st1]
        main(st1, 0)
        prep(st1)
        for it in range(1, 3):
            for k in range(NCHUNK):
                main(sts[k], it)
                if it < 2:
                    prep(sts[k])
                else:
                    squash(sts[k], k)
    nc.compile()
    return nc


_NC_CACHE = {}


def _get_nc():
    if "nc" not in _NC_CACHE:
        _NC_CACHE["nc"] = _build_bass()
    return _NC_CACHE["nc"]


def _shard_inputs(x, weight):
    # wr[m, (t, l, o)] = weight[o, l, m, i, j], t = i*3+j
    wr = np.ascontiguousarray(
        weight.transpose(2, 3, 4, 1, 0).reshape(M, NTAP * 256)
        .astype(np.float32))
    in_maps = []
    for core in range(NCORES):
        b = core // 4
        oh0 = (core % 4) * ROWS_PER_CORE
        xs = np.zeros((CIN, 10, 34), np.float32)
        lo, hi = oh0 - 1, oh0 + 9
        vlo, vhi = max(lo, 0), min(hi, H)
        xs[:, vlo - lo:vhi - lo, 1:33] = x[b, :, vlo:vhi, :]
        # j-shifted channel-major: xs2j[c, j, h, 32]
        xs2j = np.stack([xs[:, :, j:j + 32] for j in range(3)], axis=1)
        xs2j = np.ascontiguousarray(xs2j.reshape(CIN, 3 * 10 * 32))
        # j-shifted m-major: xsj[m, j, g, h, 32]
        xs_m = xs.reshape(G, M, 10, 34)
        xsj = np.stack([xs_m[:, :, :, j:j + 32] for j in range(3)], axis=2)
        # [g, m, j, h, w] -> [m, j, g, h, w]
        xsj = np.ascontiguousarray(
            xsj.transpose(1, 2, 0, 3, 4).reshape(M, 3 * G * 10 * 32))
        in_maps.append({"xs": xsj, "xs2": xs2j, "wgt": wr})
    return in_maps


def _gather_output(results):
    out = np.zeros((B, COUT, H, W), np.float32)
    for core in range(NCORES):
        b = core // 4
        oh0 = (core % 4) * ROWS_PER_CORE
        out[b, :, oh0:oh0 + ROWS_PER_CORE, :] = results[core]["out"]
    return out


def kernel(x: np.ndarray, weight: np.ndarray) -> np.ndarray:
    from concourse.bass_utils import run_bass_kernel_spmd

    x = np.asarray(x, np.float32)
    weight = np.asarray(weight, np.float32)
    res = run_bass_kernel_spmd(_get_nc(), _shard_inputs(x, weight),
                               list(range(NCORES)))
    return _gather_output(res.results)
